# revision 4
# baseline (speedup 1.0000x reference)
"""Trainium2 Bass kernel for the ClusteringLayer (vq_codebook) problem.

Computes, for x [262144, 256] f32 and clusters [512, 256] f32:
    dist2 = ||x||^2 + ||c||^2 - 2 x.c
    q = 1 / (1 + dist2)          (ALPHA == 1 makes the power a no-op)
    out = q / q.sum(axis=1, keepdims=True)

Sharding: data-parallel over N across 8 NeuronCores (32768 rows/core),
clusters replicated. No cross-core communication.

v2 changes vs the 388us baseline (which was DMA-bound at 96 MiB/core):
  - fp16 input (xt) and fp16 output (q): 48 MiB/core of HBM traffic,
    DMA roofline ~ 48 MiB / 358 GB/s ~ 140 us. fp16 rounding adds
    ~5e-4 relative error, far under the 2e-2 gate. The host quantizes
    x and c to fp16 and computes xsq/csq from the *quantized* values,
    so the kernel's dist2 is an exact ||x~ - c~||^2.
  - 2 MiB DMAs (16-super input chunks, 4-super output groups) instead
    of 256 KiB: per the DMA table, >=1 MiB reaches ~80%+ of peak vs
    ~60% at 256 KiB.
  - The [K=2, N=512] "fold" matmul (adds xsq[n] + 1 + csq[k] into
    PSUM) previously cost a full 512-cycle stream per 128-row block,
    1/3 of all PE time. Now the four folds of a super are issued
    back-to-back at tile_position (0,0)/(32,0)/(64,0)/(96,0): disjoint
    32-row strips of the PE array execute concurrently (~550 cycles
    for all four instead of 4 x 518).
  - xsq is precomputed on host (it's O(N*D), same cost class as the
    host-side transpose) and shipped as the fold lhsT rows directly:
    drops the on-device DVE squares, PE ones-reduce and ACT copy.
  - ACT does q = Reciprocal(psum) with accum_out = rowsum in one pass
    (raw InstActivation, bypassing the bass accuracy guard -- validated
    against the reference). DVE does 1/rowsum and the fp16 scale-out
    at 4x mode.
"""

import os

import numpy as np

import concourse.bass as bass
from concourse import bacc
import concourse.tile as tile
from concourse import mybir
from concourse.bass_utils import run_bass_kernel_spmd

N_TOTAL = 262144
D = 256
K = 512
N_CORES = 8
N_SHARD = N_TOTAL // N_CORES  # 32768
SUPER = 512
N_SUPERS = N_SHARD // SUPER  # 64
BLOCKS = SUPER // 128  # 4
CHUNK_SUPERS = 16  # xt load granularity: [128, 8192] fp16 = 2 MiB per d-half
N_CHUNKS = N_SUPERS // CHUNK_SUPERS  # 4
CHUNK_COLS = CHUNK_SUPERS * SUPER  # 8192
OUT_SUPERS = 4  # store granularity: [128, 8192] fp16 = 2 MiB
N_GROUPS = N_SUPERS // OUT_SUPERS  # 16
GROUP_BLOCKS = OUT_SUPERS * BLOCKS  # 16

F32 = mybir.dt.float32
F16 = mybir.dt.float16


def _r32(ap):
    """Bitcast an fp32 AP to float32r: same bits, but the PE streams it at
    1 cycle/row (vs 4 for fp32, which lowers to 2 half-speed matmuls)."""
    return ap.bitcast(mybir.dt.float32r)


def _act_raw(nc, out, in_, func, bias=0.0, scale=1.0, alpha=0.0, accum_out=None):
    """nc.scalar.activation without the Reciprocal/Rsqrt ValueError guard.

    out = func(in_ * scale + bias); accum_out (optional) = sum(out) along
    the free dim, [P, 1].
    """
    eng = nc.scalar
    inputs = [eng.lower_ap(in_)]
    for arg in (bias, scale, alpha):
        inputs.append(mybir.ImmediateValue(dtype=mybir.dt.float32, value=float(arg)))
    outputs = [eng.lower_ap(out)]
    if accum_out is not None:
        outputs.append(eng.lower_ap(accum_out))
    return eng.add_instruction(
        mybir.InstActivation(
            name=nc.get_next_instruction_name(),
            func=func,
            ins=inputs,
            outs=outputs,
        )
    )


def _build_program(n_passes=1):
    nc = bacc.Bacc()

    xt_ext = nc.declare_dram_parameter("xt", [D, N_SHARD], F16, isOutput=False)
    w_ext = nc.declare_dram_parameter("w", [D, K], F16, isOutput=False)
    frhs_ext = nc.declare_dram_parameter("frhs", [8, K], F32, isOutput=False)
    folds_ext = nc.declare_dram_parameter(
        "folds", [8, N_SUPERS * 128], F32, isOutput=False
    )
    q_ext = nc.declare_dram_parameter("q", [N_SHARD, K], F16, isOutput=True)

    ds = bass.ds
    # [2, 128, 32768]: d-chunk-major view so one DMA covers one d-half.
    xt_view = xt_ext.rearrange("(c p) n -> c p n", c=2)
    w_view = w_ext.rearrange("(c p) k -> c p k", c=2)
    # [16, 128, 16, 512]: group-major view; partition p holds, for each of
    # the 16 blocks b in the group, the K outputs of row (G*2048 + b*128 + p).
    q_view = q_ext.rearrange("(G b p) k -> G p b k", b=GROUP_BLOCKS, p=128)

    env = os.environ.get
    xt_bufs = int(env("CK_XT_BUFS", "2"))
    q_bufs = int(env("CK_Q_BUFS", "4"))
    out_bufs = int(env("CK_OUT_BUFS", "2"))
    ps_bufs = int(env("CK_PS_BUFS", "8"))
    store_eng = env("CK_STORE_ENGINE", "sync")

    with tile.TileContext(nc) as tc:
        with (
            tc.tile_pool(name="const", bufs=1) as const_pool,
            tc.tile_pool(name="xt", bufs=xt_bufs) as xt_pool,
            tc.tile_pool(name="q", bufs=q_bufs) as q_pool,
            tc.tile_pool(name="out", bufs=out_bufs) as out_pool,
            tc.tile_pool(name="small", bufs=8) as small_pool,
            tc.tile_pool(name="ps", bufs=ps_bufs, space="PSUM") as psum_pool,
        ):
            # Persistent constants
            w0 = const_pool.tile([128, K], F16, tag="w0")
            w1 = const_pool.tile([128, K], F16, tag="w1")
            # Fold operands live at partition pairs {32b, 32b+1} so the four
            # per-super folds land in disjoint 32-row PE strips and run
            # concurrently (row tiling).
            frhs4 = const_pool.tile([128, K], F32, tag="frhs4")
            fold_t = const_pool.tile([128, N_SUPERS * 128], F32, tag="fold_t")

            nc.sync.dma_start(out=w0[:], in_=w_view[0])
            nc.sync.dma_start(out=w1[:], in_=w_view[1])
            for b in range(BLOCKS):
                nc.sync.dma_start(
                    out=_r32(frhs4[32 * b : 32 * b + 2, :]),
                    in_=_r32(frhs_ext[2 * b : 2 * b + 2, :]),
                )
            for b in range(BLOCKS):
                nc.sync.dma_start(
                    out=_r32(fold_t[32 * b : 32 * b + 2, :]),
                    in_=_r32(folds_ext[2 * b : 2 * b + 2, :]),
                )

            # Warm-up chain: each dummy matmul makes the PE observe exactly
            # one new DMA semaphore, so steady-state matmuls carry at most
            # one un-observed wait (the fp32-path matmul instruction can
            # hold only a single sync wait).
            scratch = psum_pool.tile([128, K], F32, tag="ps")
            nc.tensor.matmul(
                scratch[0:1, 0:2], lhsT=w0[:, 0:1], rhs=w0[:, 0:2],
                start=True, stop=True,
            )
            nc.tensor.matmul(
                scratch[0:1, 0:2], lhsT=w1[:, 0:1], rhs=w1[:, 0:2],
                start=True, stop=True,
            )
            for b in range(BLOCKS):
                nc.tensor.matmul(
                    scratch[0:1, 0:2],
                    lhsT=_r32(frhs4[32 * b : 32 * b + 2, 0:1]),
                    rhs=_r32(frhs4[32 * b : 32 * b + 2, 0:2]),
                    start=True, stop=True, tile_position=(32 * b, 0),
                )
            for b in range(BLOCKS):
                nc.tensor.matmul(
                    scratch[0:1, 0:2],
                    lhsT=_r32(fold_t[32 * b : 32 * b + 2, 0:1]),
                    rhs=_r32(fold_t[32 * b : 32 * b + 2, 0:2]),
                    start=True, stop=True, tile_position=(32 * b, 0),
                )

            for _ in range(n_passes):
                for g in range(N_CHUNKS):
                    xt0 = xt_pool.tile([128, CHUNK_COLS], F16, tag="xt0")
                    xt1 = xt_pool.tile([128, CHUNK_COLS], F16, tag="xt1")
                    nc.sync.dma_start(
                        out=xt0[:], in_=xt_view[0, :, ds(g * CHUNK_COLS, CHUNK_COLS)]
                    )
                    nc.sync.dma_start(
                        out=xt1[:], in_=xt_view[1, :, ds(g * CHUNK_COLS, CHUNK_COLS)]
                    )
                    for so in range(CHUNK_SUPERS // OUT_SUPERS):
                        ot = out_pool.tile([128, GROUP_BLOCKS * K], F16, tag="ot")
                        for si in range(OUT_SUPERS):
                            s = (g * (CHUNK_SUPERS // OUT_SUPERS) + so) * OUT_SUPERS + si
                            col = (so * OUT_SUPERS + si) * SUPER
                            pss = []
                            # Four concurrent K=2 folds: psum <- xsq[n]*1 + 1*(1+csq[k])
                            for b in range(BLOCKS):
                                ps = psum_pool.tile([128, K], F32, tag="ps")
                                nc.tensor.matmul(
                                    ps[:],
                                    lhsT=_r32(
                                        fold_t[32 * b : 32 * b + 2, ds(s * 128, 128)]
                                    ),
                                    rhs=_r32(frhs4[32 * b : 32 * b + 2, :]),
                                    start=True, stop=False,
                                    tile_position=(32 * b, 0),
                                    skip_group_check=True,
                                )
                                pss.append(ps)
                            # Main cross-term matmuls: psum += -2 x.c
                            for b in range(BLOCKS):
                                nc.tensor.matmul(
                                    pss[b][:],
                                    lhsT=xt0[:, ds(col + b * 128, 128)],
                                    rhs=w0[:],
                                    start=False, stop=False,
                                    skip_group_check=True,
                                )
                                nc.tensor.matmul(
                                    pss[b][:],
                                    lhsT=xt1[:, ds(col + b * 128, 128)],
                                    rhs=w1[:],
                                    start=False, stop=True,
                                    skip_group_check=True,
                                )
                            for b in range(BLOCKS):
                                qt = q_pool.tile([128, K], F16, tag="qt")
                                rs = small_pool.tile([128, 1], F32, tag="rs")
                                _act_raw(
                                    nc, qt[:], pss[b][:],
                                    mybir.ActivationFunctionType.Reciprocal,
                                    accum_out=rs[:],
                                )
                                si_t = small_pool.tile([128, 1], F32, tag="si")
                                nc.vector.reciprocal(si_t[:], rs[:])
                                nc.vector.tensor_scalar(
                                    ot[:, ds((si * BLOCKS + b) * K, K)],
                                    qt[:], si_t[:], None, mybir.AluOpType.mult,
                                )
                        getattr(nc, store_eng).dma_start(
                            out=q_view[g * (CHUNK_SUPERS // OUT_SUPERS) + so],
                            in_=ot[:],
                        )

    nc.finalize()
    return nc


_PROGRAM_CACHE = {}


def _get_program(n_passes=1):
    if n_passes not in _PROGRAM_CACHE:
        _PROGRAM_CACHE[n_passes] = _build_program(n_passes)
    return _PROGRAM_CACHE[n_passes]


def _prep_inputs(x, clusters):
    x = np.ascontiguousarray(x, dtype=np.float32)
    clusters = np.ascontiguousarray(clusters, dtype=np.float32)
    # Quantize clusters to fp16 and derive w/csq from the quantized values
    # so dist2 is an exact ||x~ - c~||^2 (consistent, non-negative).
    c16 = clusters.astype(np.float16)
    c32 = c16.astype(np.float32)
    w = np.ascontiguousarray((-2.0 * c32).T).astype(np.float16)  # [D, K]
    csq1 = (1.0 + (c32 * c32).sum(axis=1)).astype(np.float32)  # [K]
    frhs = np.zeros((8, K), np.float32)
    for b in range(BLOCKS):
        frhs[2 * b] = 1.0
        frhs[2 * b + 1] = csq1
    x16 = x.astype(np.float16)
    x32 = x16.astype(np.float32)
    xsq = np.einsum("nd,nd->n", x32, x32)  # [N] f32, from the quantized x
    in_maps = []
    for i in range(N_CORES):
        sh16 = x16[i * N_SHARD : (i + 1) * N_SHARD]
        xt = np.ascontiguousarray(sh16.T)  # [D, N_SHARD] fp16
        xsq_sh = xsq[i * N_SHARD : (i + 1) * N_SHARD].reshape(N_SUPERS, BLOCKS, 128)
        folds = np.ones((8, N_SUPERS * 128), np.float32)
        for b in range(BLOCKS):
            folds[2 * b] = xsq_sh[:, b, :].reshape(-1)
        in_maps.append({"xt": xt, "w": w, "frhs": frhs, "folds": folds})
    return in_maps


def run_on_hw(x, clusters, trace=False, **kwargs):
    n_passes = int(os.environ.get("CLUSTER_KERNEL_PASSES", "1"))
    nc = _get_program(n_passes)
    in_maps = _prep_inputs(x, clusters)
    res = run_bass_kernel_spmd(
        nc, in_maps, list(range(N_CORES)), trace=trace, **kwargs
    )
    out = np.concatenate(
        [res.results[i]["q"].astype(np.float32) for i in range(N_CORES)], axis=0
    )
    return out, res


def kernel(x, clusters):
    out, _ = run_on_hw(x, clusters, trace=False)
    return out


# revision 18
# speedup vs baseline: 497.6761x; 497.6761x over previous
"""Trainium2 Bass kernel for the ClusteringLayer (vq_codebook) problem.

Computes, for x [262144, 256] f32 and clusters [512, 256] f32:
    dist2 = ||x||^2 + ||c||^2 - 2 x.c
    q = 1 / (1 + dist2)          (ALPHA == 1 makes the power a no-op)
    out = q / q.sum(axis=1, keepdims=True)

Sharding: data-parallel over N across 8 NeuronCores (32768 rows/core),
clusters replicated. No cross-core communication.

v6 design (vs the 388us f32 baseline, which was DMA-bound at 96 MiB/core):
  - fp16 input and output: 48 MiB/core HBM traffic, DMA roofline
    ~ 48 MiB / 358 GB/s ~ 140 us. Host quantizes x and c to fp16 and
    computes xsq/csq from the quantized values, so the kernel's dist2
    is an exact ||x~ - c~||^2 (rel err ~1e-3 << 2e-2 gate).
  - Work unit = group of 4 supers (2048 rows). Input: one 1 MiB DMA
    per group (both d-halves packed, contiguous 8 KB/partition) on the
    sync HWDGE ring; PE starts ~4 us into the kernel. Output: one
    2 MiB store per group in device-native layout (contiguous
    16 KB/partition; the host unpermutes) on the scalar HWDGE ring,
    decoupled from loads. The final group stores per-super to shorten
    the drain tail. Constants ride the scalar ring (2 tiles, 5 DMAs,
    all fp16) so they don't delay the first input group.
  - The xsq[n] + 1 + csq[k] term enters PSUM via K=2 fp16 fold matmuls
    whose operands sit at partition pairs {32b, 32b+1}: the four folds
    of a super are issued back-to-back at tile_position (32b, 0) so
    they execute concurrently in disjoint 32-row PE strips (~1/3 the
    serial cost; HW row-tiling, not modeled by TimelineSim).
  - xsq is precomputed on host (O(N*D), same class as the host-side
    transpose) and shipped as the fold lhsT rows: no on-device DVE
    squares / PE ones-reduce / ACT copy.
  - Per super: ONE [128, 2048] PSUM tile (4 banks) and ONE 2048-wide
    ACT Reciprocal (amortizes the ~370ns ACT instruction overhead; the
    accum_out variant would double ACT engine time). Rowsum via DVE
    tensor_scalar identity-multiply with accum_out (4x mode;
    tensor_reduce has no DVE fast mode), then 1/rowsum and the fp16
    scale-out, all on DVE.

Measured (blocked device-resident A/B slope, methodology reads the
388us baseline as ~355us): full-DVE rowsum 326 us/pass, split2 (two
rowsums on ACT-accum, two on DVE) 314 us/pass -> default split2.
A DVE-less ablation measures 222 us/pass, i.e. the post-reciprocal
normalization (rowsum + scale) costs ~90-100 us of critical path on
real HW (DVE runs far below the cost model; TimelineSim predicts
167 us/pass steady-state for this kernel).
"""

import os

import numpy as np

import concourse.bass as bass
from concourse import bacc
import concourse.tile as tile
from concourse import mybir
from concourse.bass_utils import run_bass_kernel_spmd

N_TOTAL = 262144
D = 256
K = 512
N_CORES = 8
N_SHARD = N_TOTAL // N_CORES  # 32768
SUPER = 512
N_SUPERS = N_SHARD // SUPER  # 64
BLOCKS = SUPER // 128  # 4
OUT_SUPERS = 4  # group = 4 supers: 1 MiB load, 2 MiB store
N_GROUPS = N_SUPERS // OUT_SUPERS  # 16
GROUP_COLS = OUT_SUPERS * SUPER  # 2048
GROUP_BLOCKS = OUT_SUPERS * BLOCKS  # 16
FOLD_COLS = K + N_SUPERS * 128  # frhs (512) + xsq rows (8192)

F32 = mybir.dt.float32
F16 = mybir.dt.float16


def _act_raw(nc, out, in_, func, bias=0.0, scale=1.0, alpha=0.0, accum_out=None):
    """nc.scalar.activation without the Reciprocal/Rsqrt ValueError guard.

    out = func(in_ * scale + bias); accum_out (optional) = sum(out) along
    the free dim, [P, 1].
    """
    eng = nc.scalar
    inputs = [eng.lower_ap(in_)]
    for arg in (bias, scale, alpha):
        inputs.append(mybir.ImmediateValue(dtype=mybir.dt.float32, value=float(arg)))
    outputs = [eng.lower_ap(out)]
    if accum_out is not None:
        outputs.append(eng.lower_ap(accum_out))
    return eng.add_instruction(
        mybir.InstActivation(
            name=nc.get_next_instruction_name(),
            func=func,
            ins=inputs,
            outs=outputs,
        )
    )


def _build_program(n_passes=1):
    nc = bacc.Bacc()

    # Packed x: [128, N_GROUPS, 2, GROUP_COLS] fp16 flattened; partition p of
    # group g holds d-row p (first half) then d-row p+128 (second half).
    xtp_ext = nc.declare_dram_parameter(
        "xtp", [128, N_GROUPS * 2 * GROUP_COLS], F16, isOutput=False
    )
    # w packed per partition: [:, 0:512] = -2c.T rows 0-127, [:, 512:1024]
    # rows 128-255.
    w_ext = nc.declare_dram_parameter("wp", [128, 2 * K], F16, isOutput=False)
    # Fold blob rows (partition pairs): row 2b: [ones(512) | xsq(block b)],
    # row 2b+1: [1+csq (512) | ones(8192)].
    foldc_ext = nc.declare_dram_parameter(
        "foldc", [8, FOLD_COLS], F16, isOutput=False
    )
    # Output in device-native layout: group-major, 16 KB/partition
    # contiguous; host unpermutes to [N, K].
    q_ext = nc.declare_dram_parameter(
        "q", [N_GROUPS * 128, GROUP_BLOCKS * K], F16, isOutput=True
    )

    ds = bass.ds
    xtp_view = xtp_ext.rearrange("p (g c) -> g p c", g=N_GROUPS)  # [16,128,4096]

    env = os.environ.get
    xt_bufs = int(env("CK_XT_BUFS", "4"))
    q_bufs = int(env("CK_Q_BUFS", "3"))
    out_bufs = int(env("CK_OUT_BUFS", "2"))
    ps_bufs = int(env("CK_PS_BUFS", "2"))
    store_eng = env("CK_STORE_ENGINE", "scalar")
    const_eng = env("CK_CONST_ENGINE", "scalar")
    # Timing-only ablation variants (produce WRONG output; bench use only):
    #   nodve  - ACT writes reciprocal straight into the store tile
    #   nofold - skip the fold matmuls (mains only)
    variant = env("CK_VARIANT", "split2")

    with tile.TileContext(nc) as tc:
        with (
            tc.tile_pool(name="const", bufs=1) as const_pool,
            tc.tile_pool(name="xt", bufs=xt_bufs) as xt_pool,
            tc.tile_pool(name="q", bufs=q_bufs) as q_pool,
            tc.tile_pool(name="out", bufs=out_bufs) as out_pool,
            tc.tile_pool(name="small", bufs=8) as small_pool,
            tc.tile_pool(name="ps", bufs=ps_bufs, space="PSUM") as psum_pool,
        ):
            # Persistent constants. Fold operands live at partition pairs
            # {32b, 32b+1} so the four per-super folds land in disjoint
            # 32-row PE strips and run concurrently (row tiling).
            wt = const_pool.tile([128, 2 * K], F16, tag="wt")
            foldc = const_pool.tile([128, FOLD_COLS], F16, tag="foldc")

            ceng = getattr(nc, const_eng)
            ceng.dma_start(out=wt[:], in_=w_ext[:])
            for b in range(BLOCKS):
                ceng.dma_start(
                    out=foldc[32 * b : 32 * b + 2, :],
                    in_=foldc_ext[2 * b : 2 * b + 2, :],
                )

            if env("CK_WARMUP", "0") == "1":
                # Warm-up chain (only needed for fp32-path matmuls, which
                # can carry a single sync wait; fp16 matmuls are exempt):
                # each dummy matmul makes the PE observe exactly one new
                # DMA semaphore.
                scratch = psum_pool.tile([128, BLOCKS * K], F32, tag="ps")
                nc.tensor.matmul(
                    scratch[0:1, 0:2], lhsT=wt[:, 0:1], rhs=wt[:, 0:2],
                    start=True, stop=True,
                )
                for b in range(BLOCKS):
                    nc.tensor.matmul(
                        scratch[0:1, 0:2],
                        lhsT=foldc[32 * b : 32 * b + 2, 0:1],
                        rhs=foldc[32 * b : 32 * b + 2, 0:2],
                        start=True, stop=True, tile_position=(32 * b, 0),
                    )

            for _ in range(n_passes):
                for grp in range(N_GROUPS):
                    xtg = xt_pool.tile([128, 2 * GROUP_COLS], F16, tag="xtg")
                    nc.sync.dma_start(out=xtg[:], in_=xtp_view[grp])
                    ot = out_pool.tile([128, GROUP_BLOCKS * K], F16, tag="ot")
                    for si in range(OUT_SUPERS):
                        s = grp * OUT_SUPERS + si
                        col = si * SUPER
                        # One 4-bank PSUM tile per super; block b owns the
                        # bank-aligned slice [:, b*512:(b+1)*512].
                        ps = psum_pool.tile([128, BLOCKS * K], F32, tag="ps")
                        # Four concurrent K=2 folds: psum <- xsq[n]*1 + 1*(1+csq[k])
                        if variant != "nofold":
                            for b in range(BLOCKS):
                                nc.tensor.matmul(
                                    ps[:, ds(b * K, K)],
                                    lhsT=foldc[
                                        32 * b : 32 * b + 2, ds(K + s * 128, 128)
                                    ],
                                    rhs=foldc[32 * b : 32 * b + 2, 0:K],
                                    start=True, stop=False,
                                    tile_position=(32 * b, 0),
                                    skip_group_check=True,
                                )
                        # Main cross-term matmuls: psum += -2 x.c
                        for b in range(BLOCKS):
                            nc.tensor.matmul(
                                ps[:, ds(b * K, K)],
                                lhsT=xtg[:, ds(col + b * 128, 128)],
                                rhs=wt[:, 0:K],
                                start=(variant == "nofold"), stop=False,
                                skip_group_check=True,
                            )
                            nc.tensor.matmul(
                                ps[:, ds(b * K, K)],
                                lhsT=xtg[:, ds(GROUP_COLS + col + b * 128, 128)],
                                rhs=wt[:, K : 2 * K],
                                start=False, stop=True,
                                skip_group_check=True,
                            )
                        # One 2048-wide reciprocal per super (amortizes the
                        # ~370ns ACT instruction overhead 4x).
                        if variant == "nodve":
                            _act_raw(
                                nc, ot[:, ds(si * BLOCKS * K, BLOCKS * K)],
                                ps[:],
                                mybir.ActivationFunctionType.Reciprocal,
                            )
                            continue
                        qt = q_pool.tile([128, BLOCKS * K], F16, tag="qt")
                        rss = [None] * BLOCKS
                        if variant == "acta":
                            # All rowsums on ACT: per-block reciprocal+accum.
                            for b in range(BLOCKS):
                                rss[b] = small_pool.tile([128, 1], F32, tag="rs", name="rs_acta")
                                _act_raw(
                                    nc, qt[:, ds(b * K, K)], ps[:, ds(b * K, K)],
                                    mybir.ActivationFunctionType.Reciprocal,
                                    accum_out=rss[b][:],
                                )
                        elif variant.startswith("split"):
                            # First `nact` blocks: rowsum on ACT (512-wide
                            # recip+accum); the rest share one wide recip,
                            # rowsums on DVE. Balances real ACT/DVE rates.
                            nact = int(variant[5:])
                            for b in range(nact):
                                rss[b] = small_pool.tile(
                                    [128, 1], F32, tag="rs", name="rs_split"
                                )
                                _act_raw(
                                    nc, qt[:, ds(b * K, K)], ps[:, ds(b * K, K)],
                                    mybir.ActivationFunctionType.Reciprocal,
                                    accum_out=rss[b][:],
                                )
                            _act_raw(
                                nc, qt[:, nact * K :], ps[:, nact * K :],
                                mybir.ActivationFunctionType.Reciprocal,
                            )
                        else:
                            _act_raw(
                                nc, qt[:], ps[:],
                                mybir.ActivationFunctionType.Reciprocal,
                            )
                        for b in range(BLOCKS):
                            if rss[b] is None:
                                qs = q_pool.tile([128, K], F16, tag="qs")
                                rs = small_pool.tile([128, 1], F32, tag="rs")
                                # Identity multiply with accumulate: rowsum on
                                # DVE (tensor_reduce has no fast mode).
                                nc.vector.tensor_scalar(
                                    qs[:], qt[:, ds(b * K, K)], 1.0, 0.0,
                                    mybir.AluOpType.mult, mybir.AluOpType.add,
                                    accum_out=rs[:],
                                )
                            else:
                                rs = rss[b]
                            si_t = small_pool.tile([128, 1], F32, tag="si")
                            nc.vector.reciprocal(si_t[:], rs[:])
                            nc.vector.tensor_scalar(
                                ot[:, ds((si * BLOCKS + b) * K, K)],
                                qt[:, ds(b * K, K)], si_t[:], None,
                                mybir.AluOpType.mult,
                            )
                        if grp == N_GROUPS - 1:
                            # Final group: store per super so the last
                            # DVE->store->drain tail is short.
                            getattr(nc, store_eng).dma_start(
                                out=q_ext[
                                    ds(grp * 128, 128),
                                    ds(si * BLOCKS * K, BLOCKS * K),
                                ],
                                in_=ot[:, ds(si * BLOCKS * K, BLOCKS * K)],
                            )
                    if grp < N_GROUPS - 1:
                        getattr(nc, store_eng).dma_start(
                            out=q_ext[ds(grp * 128, 128), :], in_=ot[:]
                        )

    nc.finalize()
    return nc


_PROGRAM_CACHE = {}


def _get_program(n_passes=1):
    if n_passes not in _PROGRAM_CACHE:
        _PROGRAM_CACHE[n_passes] = _build_program(n_passes)
    return _PROGRAM_CACHE[n_passes]


def _prep_inputs(x, clusters):
    x = np.ascontiguousarray(x, dtype=np.float32)
    clusters = np.ascontiguousarray(clusters, dtype=np.float32)
    # Quantize clusters to fp16 and derive w/csq from the quantized values
    # so dist2 is an exact ||x~ - c~||^2 (consistent, non-negative).
    c16 = clusters.astype(np.float16)
    c32 = c16.astype(np.float32)
    w = (-2.0 * c32).T.astype(np.float16)  # [D, K]
    wp = np.ascontiguousarray(
        w.reshape(2, 128, K).transpose(1, 0, 2).reshape(128, 2 * K)
    )
    csq1 = (1.0 + (c32 * c32).sum(axis=1)).astype(np.float16)  # [K]
    x16 = x.astype(np.float16)
    x32 = x16.astype(np.float32)
    xsq = np.einsum("nd,nd->n", x32, x32).astype(np.float16)
    in_maps = []
    for i in range(N_CORES):
        sh16 = x16[i * N_SHARD : (i + 1) * N_SHARD]
        xt = sh16.T  # [D, N_SHARD] view
        # [128, g, c, j]: partition p, group g, d-half c, column j
        xtp = np.ascontiguousarray(
            xt.reshape(2, 128, N_GROUPS, GROUP_COLS).transpose(1, 2, 0, 3)
        ).reshape(128, N_GROUPS * 2 * GROUP_COLS)
        xsq_sh = (
            xsq[i * N_SHARD : (i + 1) * N_SHARD]
            .reshape(N_SUPERS, BLOCKS, 128)
        )
        foldc = np.ones((8, FOLD_COLS), np.float16)
        for b in range(BLOCKS):
            foldc[2 * b, K:] = xsq_sh[:, b, :].reshape(-1)
            foldc[2 * b + 1, :K] = csq1
        in_maps.append({"xtp": xtp, "wp": wp, "foldc": foldc})
    return in_maps


def run_on_hw(x, clusters, trace=False, **kwargs):
    n_passes = int(os.environ.get("CLUSTER_KERNEL_PASSES", "1"))
    nc = _get_program(n_passes)
    in_maps = _prep_inputs(x, clusters)
    res = run_bass_kernel_spmd(
        nc, in_maps, list(range(N_CORES)), trace=trace, **kwargs
    )
    outs = []
    for i in range(N_CORES):
        qdev = res.results[i]["q"]  # [N_GROUPS*128, GROUP_BLOCKS*K] fp16
        # [g, p, b, k] -> row g*2048 + b*128 + p
        qsh = (
            qdev.reshape(N_GROUPS, 128, GROUP_BLOCKS, K)
            .transpose(0, 2, 1, 3)
            .reshape(N_SHARD, K)
        )
        outs.append(qsh.astype(np.float32))
    out = np.concatenate(outs, axis=0)
    return out, res


def kernel(x, clusters):
    out, _ = run_on_hw(x, clusters, trace=False)
    return out


# revision 19
# speedup vs baseline: 1342.1556x; 2.6968x over previous
"""Trainium2 Bass kernel for the ClusteringLayer (vq_codebook) problem.

Computes, for x [262144, 256] f32 and clusters [512, 256] f32:
    dist2 = ||x||^2 + ||c||^2 - 2 x.c
    q = 1 / (1 + dist2)          (ALPHA == 1 makes the power a no-op)
    out = q / q.sum(axis=1, keepdims=True)

Sharding: data-parallel over N across 8 NeuronCores (32768 rows/core),
clusters replicated. No cross-core communication.

v6 design (vs the 388us f32 baseline, which was DMA-bound at 96 MiB/core):
  - fp16 input and output: 48 MiB/core HBM traffic, DMA roofline
    ~ 48 MiB / 358 GB/s ~ 140 us. Host quantizes x and c to fp16 and
    computes xsq/csq from the quantized values, so the kernel's dist2
    is an exact ||x~ - c~||^2 (rel err ~1e-3 << 2e-2 gate).
  - Work unit = group of 4 supers (2048 rows). Input: one 1 MiB DMA
    per group (both d-halves packed, contiguous 8 KB/partition) on the
    sync HWDGE ring; PE starts ~4 us into the kernel. Output: one
    2 MiB store per group in device-native layout (contiguous
    16 KB/partition; the host unpermutes) on the scalar HWDGE ring,
    decoupled from loads. The final group stores per-super to shorten
    the drain tail. Constants ride the scalar ring (2 tiles, 5 DMAs,
    all fp16) so they don't delay the first input group.
  - The xsq[n] + 1 + csq[k] term enters PSUM via K=2 fp16 fold matmuls
    whose operands sit at partition pairs {32b, 32b+1}: the four folds
    of a super are issued back-to-back at tile_position (32b, 0) so
    they execute concurrently in disjoint 32-row PE strips (~1/3 the
    serial cost; HW row-tiling, not modeled by TimelineSim).
  - xsq is precomputed on host (O(N*D), same class as the host-side
    transpose) and shipped as the fold lhsT rows: no on-device DVE
    squares / PE ones-reduce / ACT copy.
  - Per super: ONE [128, 2048] PSUM tile (4 banks) and ONE 2048-wide
    ACT Reciprocal (amortizes the ~370ns ACT instruction overhead; the
    accum_out variant would double ACT engine time). Rowsum via DVE
    tensor_scalar identity-multiply with accum_out (4x mode;
    tensor_reduce has no DVE fast mode), then 1/rowsum and the fp16
    scale-out, all on DVE.

Measured (blocked device-resident A/B slope on a quiet device):
split2 = 164 us/pass, below its TimelineSim steady-state (~198 us),
consistent with HW fold row-tiling packing that the cost model does
not simulate. TimelineSim single-shot: full 190, split1 191,
split2 218, acta 237 -> default split1 (sim-minimal like full, and
hedges the risk of DVE accum running slow by keeping one rowsum per
super on ACT). Earlier contention-era measurements (full 326 /
split2 314 / nodve 222) were taken under heavy shared-device load.
"""

import os

import numpy as np

import concourse.bass as bass
from concourse import bacc
import concourse.tile as tile
from concourse import mybir
from concourse.bass_utils import run_bass_kernel_spmd

N_TOTAL = 262144
D = 256
K = 512
N_CORES = 8
N_SHARD = N_TOTAL // N_CORES  # 32768
SUPER = 512
N_SUPERS = N_SHARD // SUPER  # 64
BLOCKS = SUPER // 128  # 4
OUT_SUPERS = 4  # group = 4 supers: 1 MiB load, 2 MiB store
N_GROUPS = N_SUPERS // OUT_SUPERS  # 16
GROUP_COLS = OUT_SUPERS * SUPER  # 2048
GROUP_BLOCKS = OUT_SUPERS * BLOCKS  # 16
FOLD_COLS = K + N_SUPERS * 128  # frhs (512) + xsq rows (8192)

F32 = mybir.dt.float32
F16 = mybir.dt.float16


def _act_raw(nc, out, in_, func, bias=0.0, scale=1.0, alpha=0.0, accum_out=None):
    """nc.scalar.activation without the Reciprocal/Rsqrt ValueError guard.

    out = func(in_ * scale + bias); accum_out (optional) = sum(out) along
    the free dim, [P, 1].
    """
    eng = nc.scalar
    inputs = [eng.lower_ap(in_)]
    for arg in (bias, scale, alpha):
        inputs.append(mybir.ImmediateValue(dtype=mybir.dt.float32, value=float(arg)))
    outputs = [eng.lower_ap(out)]
    if accum_out is not None:
        outputs.append(eng.lower_ap(accum_out))
    return eng.add_instruction(
        mybir.InstActivation(
            name=nc.get_next_instruction_name(),
            func=func,
            ins=inputs,
            outs=outputs,
        )
    )


def _build_program(n_passes=1):
    nc = bacc.Bacc()

    # Packed x: [128, N_GROUPS, 2, GROUP_COLS] fp16 flattened; partition p of
    # group g holds d-row p (first half) then d-row p+128 (second half).
    xtp_ext = nc.declare_dram_parameter(
        "xtp", [128, N_GROUPS * 2 * GROUP_COLS], F16, isOutput=False
    )
    # w packed per partition: [:, 0:512] = -2c.T rows 0-127, [:, 512:1024]
    # rows 128-255.
    w_ext = nc.declare_dram_parameter("wp", [128, 2 * K], F16, isOutput=False)
    # Fold blob rows (partition pairs): row 2b: [ones(512) | xsq(block b)],
    # row 2b+1: [1+csq (512) | ones(8192)].
    foldc_ext = nc.declare_dram_parameter(
        "foldc", [8, FOLD_COLS], F16, isOutput=False
    )
    # Output in device-native layout: group-major, 16 KB/partition
    # contiguous; host unpermutes to [N, K].
    q_ext = nc.declare_dram_parameter(
        "q", [N_GROUPS * 128, GROUP_BLOCKS * K], F16, isOutput=True
    )

    ds = bass.ds
    xtp_view = xtp_ext.rearrange("p (g c) -> g p c", g=N_GROUPS)  # [16,128,4096]

    env = os.environ.get
    xt_bufs = int(env("CK_XT_BUFS", "4"))
    q_bufs = int(env("CK_Q_BUFS", "3"))
    out_bufs = int(env("CK_OUT_BUFS", "2"))
    ps_bufs = int(env("CK_PS_BUFS", "2"))
    store_eng = env("CK_STORE_ENGINE", "scalar")
    const_eng = env("CK_CONST_ENGINE", "scalar")
    # Timing-only ablation variants (produce WRONG output; bench use only):
    #   nodve  - ACT writes reciprocal straight into the store tile
    #   nofold - skip the fold matmuls (mains only)
    variant = env("CK_VARIANT", "split1")

    with tile.TileContext(nc) as tc:
        with (
            tc.tile_pool(name="const", bufs=1) as const_pool,
            tc.tile_pool(name="xt", bufs=xt_bufs) as xt_pool,
            tc.tile_pool(name="q", bufs=q_bufs) as q_pool,
            tc.tile_pool(name="out", bufs=out_bufs) as out_pool,
            tc.tile_pool(name="small", bufs=8) as small_pool,
            tc.tile_pool(name="ps", bufs=ps_bufs, space="PSUM") as psum_pool,
        ):
            # Persistent constants. Fold operands live at partition pairs
            # {32b, 32b+1} so the four per-super folds land in disjoint
            # 32-row PE strips and run concurrently (row tiling).
            wt = const_pool.tile([128, 2 * K], F16, tag="wt")
            foldc = const_pool.tile([128, FOLD_COLS], F16, tag="foldc")

            ceng = getattr(nc, const_eng)
            ceng.dma_start(out=wt[:], in_=w_ext[:])
            for b in range(BLOCKS):
                ceng.dma_start(
                    out=foldc[32 * b : 32 * b + 2, :],
                    in_=foldc_ext[2 * b : 2 * b + 2, :],
                )

            if env("CK_WARMUP", "0") == "1":
                # Warm-up chain (only needed for fp32-path matmuls, which
                # can carry a single sync wait; fp16 matmuls are exempt):
                # each dummy matmul makes the PE observe exactly one new
                # DMA semaphore.
                scratch = psum_pool.tile([128, BLOCKS * K], F32, tag="ps")
                nc.tensor.matmul(
                    scratch[0:1, 0:2], lhsT=wt[:, 0:1], rhs=wt[:, 0:2],
                    start=True, stop=True,
                )
                for b in range(BLOCKS):
                    nc.tensor.matmul(
                        scratch[0:1, 0:2],
                        lhsT=foldc[32 * b : 32 * b + 2, 0:1],
                        rhs=foldc[32 * b : 32 * b + 2, 0:2],
                        start=True, stop=True, tile_position=(32 * b, 0),
                    )

            for _ in range(n_passes):
                for grp in range(N_GROUPS):
                    xtg = xt_pool.tile([128, 2 * GROUP_COLS], F16, tag="xtg")
                    nc.sync.dma_start(out=xtg[:], in_=xtp_view[grp])
                    ot = out_pool.tile([128, GROUP_BLOCKS * K], F16, tag="ot")
                    for si in range(OUT_SUPERS):
                        s = grp * OUT_SUPERS + si
                        col = si * SUPER
                        # One 4-bank PSUM tile per super; block b owns the
                        # bank-aligned slice [:, b*512:(b+1)*512].
                        ps = psum_pool.tile([128, BLOCKS * K], F32, tag="ps")
                        # Four concurrent K=2 folds: psum <- xsq[n]*1 + 1*(1+csq[k])
                        if variant != "nofold":
                            for b in range(BLOCKS):
                                nc.tensor.matmul(
                                    ps[:, ds(b * K, K)],
                                    lhsT=foldc[
                                        32 * b : 32 * b + 2, ds(K + s * 128, 128)
                                    ],
                                    rhs=foldc[32 * b : 32 * b + 2, 0:K],
                                    start=True, stop=False,
                                    tile_position=(32 * b, 0),
                                    skip_group_check=True,
                                )
                        # Main cross-term matmuls: psum += -2 x.c
                        for b in range(BLOCKS):
                            nc.tensor.matmul(
                                ps[:, ds(b * K, K)],
                                lhsT=xtg[:, ds(col + b * 128, 128)],
                                rhs=wt[:, 0:K],
                                start=(variant == "nofold"), stop=False,
                                skip_group_check=True,
                            )
                            nc.tensor.matmul(
                                ps[:, ds(b * K, K)],
                                lhsT=xtg[:, ds(GROUP_COLS + col + b * 128, 128)],
                                rhs=wt[:, K : 2 * K],
                                start=False, stop=True,
                                skip_group_check=True,
                            )
                        # One 2048-wide reciprocal per super (amortizes the
                        # ~370ns ACT instruction overhead 4x).
                        if variant == "nodve":
                            _act_raw(
                                nc, ot[:, ds(si * BLOCKS * K, BLOCKS * K)],
                                ps[:],
                                mybir.ActivationFunctionType.Reciprocal,
                            )
                            continue
                        qt = q_pool.tile([128, BLOCKS * K], F16, tag="qt")
                        rss = [None] * BLOCKS
                        if variant == "acta":
                            # All rowsums on ACT: per-block reciprocal+accum.
                            for b in range(BLOCKS):
                                rss[b] = small_pool.tile([128, 1], F32, tag="rs", name="rs_acta")
                                _act_raw(
                                    nc, qt[:, ds(b * K, K)], ps[:, ds(b * K, K)],
                                    mybir.ActivationFunctionType.Reciprocal,
                                    accum_out=rss[b][:],
                                )
                        elif variant.startswith("split"):
                            # First `nact` blocks: rowsum on ACT (512-wide
                            # recip+accum); the rest share one wide recip,
                            # rowsums on DVE. Balances real ACT/DVE rates.
                            nact = int(variant[5:])
                            for b in range(nact):
                                rss[b] = small_pool.tile(
                                    [128, 1], F32, tag="rs", name="rs_split"
                                )
                                _act_raw(
                                    nc, qt[:, ds(b * K, K)], ps[:, ds(b * K, K)],
                                    mybir.ActivationFunctionType.Reciprocal,
                                    accum_out=rss[b][:],
                                )
                            _act_raw(
                                nc, qt[:, nact * K :], ps[:, nact * K :],
                                mybir.ActivationFunctionType.Reciprocal,
                            )
                        else:
                            _act_raw(
                                nc, qt[:], ps[:],
                                mybir.ActivationFunctionType.Reciprocal,
                            )
                        for b in range(BLOCKS):
                            if rss[b] is None:
                                qs = q_pool.tile([128, K], F16, tag="qs")
                                rs = small_pool.tile([128, 1], F32, tag="rs")
                                # Identity multiply with accumulate: rowsum on
                                # DVE (tensor_reduce has no fast mode).
                                nc.vector.tensor_scalar(
                                    qs[:], qt[:, ds(b * K, K)], 1.0, 0.0,
                                    mybir.AluOpType.mult, mybir.AluOpType.add,
                                    accum_out=rs[:],
                                )
                            else:
                                rs = rss[b]
                            si_t = small_pool.tile([128, 1], F32, tag="si")
                            nc.vector.reciprocal(si_t[:], rs[:])
                            nc.vector.tensor_scalar(
                                ot[:, ds((si * BLOCKS + b) * K, K)],
                                qt[:, ds(b * K, K)], si_t[:], None,
                                mybir.AluOpType.mult,
                            )
                        if grp == N_GROUPS - 1:
                            # Final group: store per super so the last
                            # DVE->store->drain tail is short.
                            getattr(nc, store_eng).dma_start(
                                out=q_ext[
                                    ds(grp * 128, 128),
                                    ds(si * BLOCKS * K, BLOCKS * K),
                                ],
                                in_=ot[:, ds(si * BLOCKS * K, BLOCKS * K)],
                            )
                    if grp < N_GROUPS - 1:
                        getattr(nc, store_eng).dma_start(
                            out=q_ext[ds(grp * 128, 128), :], in_=ot[:]
                        )

    nc.finalize()
    return nc


_PROGRAM_CACHE = {}


def _get_program(n_passes=1):
    if n_passes not in _PROGRAM_CACHE:
        _PROGRAM_CACHE[n_passes] = _build_program(n_passes)
    return _PROGRAM_CACHE[n_passes]


def _prep_inputs(x, clusters):
    x = np.ascontiguousarray(x, dtype=np.float32)
    clusters = np.ascontiguousarray(clusters, dtype=np.float32)
    # Quantize clusters to fp16 and derive w/csq from the quantized values
    # so dist2 is an exact ||x~ - c~||^2 (consistent, non-negative).
    c16 = clusters.astype(np.float16)
    c32 = c16.astype(np.float32)
    w = (-2.0 * c32).T.astype(np.float16)  # [D, K]
    wp = np.ascontiguousarray(
        w.reshape(2, 128, K).transpose(1, 0, 2).reshape(128, 2 * K)
    )
    csq1 = (1.0 + (c32 * c32).sum(axis=1)).astype(np.float16)  # [K]
    x16 = x.astype(np.float16)
    x32 = x16.astype(np.float32)
    xsq = np.einsum("nd,nd->n", x32, x32).astype(np.float16)
    in_maps = []
    for i in range(N_CORES):
        sh16 = x16[i * N_SHARD : (i + 1) * N_SHARD]
        xt = sh16.T  # [D, N_SHARD] view
        # [128, g, c, j]: partition p, group g, d-half c, column j
        xtp = np.ascontiguousarray(
            xt.reshape(2, 128, N_GROUPS, GROUP_COLS).transpose(1, 2, 0, 3)
        ).reshape(128, N_GROUPS * 2 * GROUP_COLS)
        xsq_sh = (
            xsq[i * N_SHARD : (i + 1) * N_SHARD]
            .reshape(N_SUPERS, BLOCKS, 128)
        )
        foldc = np.ones((8, FOLD_COLS), np.float16)
        for b in range(BLOCKS):
            foldc[2 * b, K:] = xsq_sh[:, b, :].reshape(-1)
            foldc[2 * b + 1, :K] = csq1
        in_maps.append({"xtp": xtp, "wp": wp, "foldc": foldc})
    return in_maps


def run_on_hw(x, clusters, trace=False, **kwargs):
    n_passes = int(os.environ.get("CLUSTER_KERNEL_PASSES", "1"))
    nc = _get_program(n_passes)
    in_maps = _prep_inputs(x, clusters)
    res = run_bass_kernel_spmd(
        nc, in_maps, list(range(N_CORES)), trace=trace, **kwargs
    )
    outs = []
    for i in range(N_CORES):
        qdev = res.results[i]["q"]  # [N_GROUPS*128, GROUP_BLOCKS*K] fp16
        # [g, p, b, k] -> row g*2048 + b*128 + p
        qsh = (
            qdev.reshape(N_GROUPS, 128, GROUP_BLOCKS, K)
            .transpose(0, 2, 1, 3)
            .reshape(N_SHARD, K)
        )
        outs.append(qsh.astype(np.float32))
    out = np.concatenate(outs, axis=0)
    return out, res


def kernel(x, clusters):
    out, _ = run_on_hw(x, clusters, trace=False)
    return out
